# revision 10
# baseline (speedup 1.0000x reference)
"""Conv2d(256->256, 3x3, pad 1) on (1,256,512,512) fp32, H-sharded over 8 TRN2 cores.

Strategy: shard output rows (H) across 8 cores, 64 rows each. Host pre-pads
the input spatially (H and W by 1) and casts to fp16, so each core receives
a clean [256, 66, 514] fp16 slice with halo rows baked in -- no device-side
boundary handling. The whole slice fits in SBUF (~136 KB/partition), loaded
once in 3 pieces per channel-chunk (3/7/56 rows) so the first matmul only
waits for weights + 3 rows. The conv is 9 shifted matmuls per output row:
  out[co, h, :] = sum_{tap,(ci chunk)} W_tap[ci,co].T @ x[ci, h+kh, kw:kw+512]
accumulated in PSUM over 18 fp16 matmuls (9 taps x 2 ci chunks of 128) per
each of 2 co chunks; fp16 runs at full PE rate (216 ns/MM incl. hidden FWL
weight loads) with fp32 PSUM accumulation, ~3e-4 rel err vs the fp32
reference. Each PSUM bank drains via DVE copy to SBUF, then a per-row DMA
writes fp32 output to HBM.
"""

import hashlib
import os
import shutil
import threading

import numpy as np

import concourse.bacc as bacc
import concourse.bass2jax as bass2jax
import concourse.tile as tile
from concourse import mybir
from concourse.bass_utils import run_bass_kernel_spmd

f32 = mybir.dt.float32
f16 = mybir.dt.float16

# The bass_exec compile path (bass2jax.neuronx_cc_hook -> compile_bir_kernel)
# has no cache, so every fresh process pays a multi-minute walrus compile of
# the identical BIR. Memoize the NEFF on disk keyed by SHA-256 of the exact
# BIR bytes (the compile is a pure function of them; the per-run tensor
# rename happens downstream of this hook).
_NEFF_CACHE = os.path.join(os.path.expanduser("~"), ".bass-neff-cache")


def _install_neff_cache():
    orig = getattr(bass2jax, "compile_bir_kernel", None)
    if orig is None or getattr(orig, "_neff_cached", False):
        return

    def cached(bir_json, tmpdir, neff_name="file.neff"):
        cpath = None
        try:
            raw = bir_json if isinstance(bir_json, bytes) else bir_json.encode()
            # The BIR embeds this file's absolute path in per-instruction
            # debug info; normalize it so the cache key is independent of
            # where kernel.py lives.
            raw = raw.replace(os.path.abspath(__file__).encode(), b"@KERNEL@")
            cpath = os.path.join(_NEFF_CACHE,
                                 hashlib.sha256(raw).hexdigest() + ".neff")
            if os.path.exists(cpath):
                dst = os.path.join(tmpdir, neff_name)
                shutil.copyfile(cpath, dst)
                return dst
        except Exception:
            cpath = None
        out = orig(bir_json, tmpdir, neff_name)
        if cpath:
            try:
                os.makedirs(_NEFF_CACHE, exist_ok=True)
                tmp = f"{cpath}.tmp{os.getpid()}"
                shutil.copyfile(out, tmp)
                os.replace(tmp, cpath)
            except Exception:
                pass
        return out

    cached._neff_cached = True
    bass2jax.compile_bir_kernel = cached


_install_neff_cache()


def _in_clean_thread(fn):
    """Run fn on a fresh thread so the Python stack (which bass embeds as
    ant_traceback debug info in the BIR) contains no caller frames -- the
    BIR, and therefore the NEFF cache key, become independent of whichever
    script invoked kernel()."""
    res = {}

    def runner():
        try:
            res["v"] = fn()
        except BaseException as e:  # propagate to caller
            res["e"] = e

    t = threading.Thread(target=runner, name="convkernel")
    t.start()
    t.join()
    if "e" in res:
        raise res["e"]
    return res["v"]


NCORES = 8
CIN = 256
COUT = 256
H = 512
W = 512
KH = KW = 3
PC = 128                 # partition chunk
NCI = CIN // PC          # 2 input-channel chunks
NCO = COUT // PC         # 2 output-channel chunks
HB = H // NCORES         # 64 output rows per core
HIN = HB + 2             # 66 input rows incl. halo
WP = W + 2               # 514 padded width
NTAP = KH * KW
XSPLITS = (3, 7, HIN - 10)   # input-row DMA pieces: head / mid / tail

_nc_cache = {}


def _build(repeats=1):
    nc = bacc.Bacc("TRN2", target_bir_lowering=False, debug=False,
                   num_devices=NCORES)
    xs = nc.dram_tensor("xs", [CIN, HIN, WP], f16, kind="ExternalInput").ap()
    wt = nc.dram_tensor("wt", [NTAP, CIN, COUT], f16, kind="ExternalInput").ap()
    out = nc.dram_tensor("out", [COUT, HB, W], f32, kind="ExternalOutput").ap()

    with tile.TileContext(nc) as tc:
        with tc.tile_pool(name="wpool", bufs=1) as wpool, \
             tc.tile_pool(name="xpool", bufs=1) as xpool, \
             tc.tile_pool(name="opool", bufs=8) as opool, \
             tc.tile_pool(name="pspool", bufs=8, space="PSUM") as pspool:

            # Warm the PE clock gate (HAM) with throwaway matmuls on a
            # memset tile while the input DMAs are in flight, so the real
            # matmul stream starts at 2.4 GHz instead of 1.2.
            warm_src = wpool.tile([PC, PC], f16, name="warm_src")
            nc.vector.memset(warm_src[:], 0.0)
            warm_ps = pspool.tile([PC, PC], f32, tag="ps", name="warm_ps")
            for i in range(36):
                nc.tensor.matmul(warm_ps[:], warm_src[:], warm_src[:],
                                 start=True, stop=True)

            # Weights [128 ci, 9 tap, 2 ci-chunk, 256 co] fp16, co-chunk 0
            # first: the first row-group only needs co=0.
            w_all = wpool.tile([PC, NTAP, NCI, COUT], f16, name="w_all")
            wt_r = wt.rearrange("t (c p) o -> p t c o", p=PC)
            nc.sync.dma_start(w_all[:, :, :, 0:PC], wt_r[:, :, :, 0:PC])

            # Whole input slice resident in SBUF, loaded head-first so the
            # first row-group starts after ~2 MB instead of ~18 MB. Issue
            # order is criticality order: both chunks' heads before mids
            # before tails, so small early-needed DMAs aren't queued behind
            # the 14 MB tails.
            x_sb = [[] for _ in range(NCI)]  # x_sb[c] = [(tile, row0, nrows)]
            r0s = np.cumsum((0,) + XSPLITS)
            for i, nr in enumerate(XSPLITS):
                r0 = int(r0s[i])
                if i == 1:
                    nc.sync.dma_start(w_all[:, :, :, PC:COUT],
                                      wt_r[:, :, :, PC:COUT])
                for c in range(NCI):
                    xt = xpool.tile([PC, nr, WP], f16, tag=f"x{c}_{i}",
                                    name=f"x{c}_{i}")
                    nc.sync.dma_start(
                        xt[:], xs[c * PC:(c + 1) * PC, r0:r0 + nr, :])
                    x_sb[c].append((xt, r0, nr))

            def row_ap(c, rr):
                for xt, r0, nr in x_sb[c]:
                    if rr < r0 + nr:
                        return xt[:, rr - r0, :]
                raise AssertionError(rr)

            for _rep in range(repeats):
                for h in range(HB):
                    for co in range(NCO):
                        ps = pspool.tile([PC, W], f32, tag="ps",
                                         name=f"ps_{h}_{co}")
                        idx = 0
                        for t in range(NTAP):
                            kh, kw = divmod(t, KW)
                            for c in range(NCI):
                                nc.tensor.matmul(
                                    ps[:],
                                    w_all[:, t, c, co * PC:(co + 1) * PC],
                                    row_ap(c, h + kh)[:, kw:kw + W],
                                    start=(idx == 0),
                                    stop=(idx == NTAP * NCI - 1))
                                idx += 1
                        o_t = opool.tile([PC, W], f32, tag="orow",
                                         name=f"o_{h}_{co}")
                        nc.vector.tensor_copy(o_t[:], ps[:])
                        nc.sync.dma_start(
                            out[co * PC:(co + 1) * PC, h, :], o_t[:])
    nc.compile()
    return nc


def _get_nc(repeats=1):
    if repeats not in _nc_cache:
        _nc_cache[repeats] = _in_clean_thread(lambda: _build(repeats))
    return _nc_cache[repeats]


def _make_in_maps(x, weight):
    x_f16 = np.asarray(x[0], dtype=np.float16)
    x_pad = np.pad(x_f16, ((0, 0), (1, 1), (1, 1)))
    w_t = np.ascontiguousarray(
        weight.transpose(2, 3, 1, 0).reshape(NTAP, CIN, COUT).astype(
            np.float16))
    in_maps = []
    for core in range(NCORES):
        xsl = np.ascontiguousarray(x_pad[:, core * HB:core * HB + HIN, :])
        in_maps.append({"xs": xsl, "wt": w_t})
    return in_maps


def kernel(x, weight):
    x = np.asarray(x, dtype=np.float32)
    weight = np.asarray(weight, dtype=np.float32)
    nc = _get_nc(1)
    in_maps = _make_in_maps(x, weight)
    res = _in_clean_thread(lambda: run_bass_kernel_spmd(
        nc, in_maps, core_ids=list(range(NCORES))))
    parts = [res.results[c]["out"] for c in range(NCORES)]
    full = np.concatenate(parts, axis=1)          # [COUT, H, W]
    return full[None].astype(np.float32)


# revision 12
# speedup vs baseline: 1.0042x; 1.0042x over previous
"""Conv2d(256->256, 3x3, pad 1) on (1,256,512,512) fp32, H-sharded over 8 TRN2 cores.

Strategy: shard output rows (H) across 8 cores, 64 rows each. Host pre-pads
the input spatially (H and W by 1) and casts to fp16, so each core receives
a clean [256, 66, 514] fp16 slice with halo rows baked in -- no device-side
boundary handling. The whole slice fits in SBUF (~136 KB/partition), loaded
once in 3 pieces per channel-chunk (3/7/56 rows) so the first matmul only
waits for weights + 3 rows. The conv is 9 shifted matmuls per output row:
  out[co, h, :] = sum_{tap,(ci chunk)} W_tap[ci,co].T @ x[ci, h+kh, kw:kw+512]
accumulated in PSUM over 18 fp16 matmuls (9 taps x 2 ci chunks of 128) per
each of 2 co chunks; fp16 runs at full PE rate (216 ns/MM incl. hidden FWL
weight loads) with fp32 PSUM accumulation, ~3e-4 rel err vs the fp32
reference. Each PSUM bank drains via DVE copy to SBUF, then a per-row DMA
writes fp32 output to HBM.
"""

import hashlib
import os
import shutil
import threading

import numpy as np

import concourse.bacc as bacc
import concourse.bass2jax as bass2jax
import concourse.tile as tile
from concourse import mybir
from concourse.bass_utils import run_bass_kernel_spmd

f32 = mybir.dt.float32
f16 = mybir.dt.float16

# The bass_exec compile path (bass2jax.neuronx_cc_hook -> compile_bir_kernel)
# has no cache, so every fresh process pays a multi-minute walrus compile of
# the identical BIR. Memoize the NEFF on disk keyed by SHA-256 of the exact
# BIR bytes (the compile is a pure function of them; the per-run tensor
# rename happens downstream of this hook).
_NEFF_CACHE = os.path.join(os.path.expanduser("~"), ".bass-neff-cache")


def _install_neff_cache():
    orig = getattr(bass2jax, "compile_bir_kernel", None)
    if orig is None or getattr(orig, "_neff_cached", False):
        return

    def cached(bir_json, tmpdir, neff_name="file.neff"):
        cpath = None
        try:
            raw = bir_json if isinstance(bir_json, bytes) else bir_json.encode()
            # The BIR embeds this file's absolute path in per-instruction
            # debug info; normalize it so the cache key is independent of
            # where kernel.py lives.
            raw = raw.replace(os.path.abspath(__file__).encode(), b"@KERNEL@")
            cpath = os.path.join(_NEFF_CACHE,
                                 hashlib.sha256(raw).hexdigest() + ".neff")
            if os.path.exists(cpath):
                dst = os.path.join(tmpdir, neff_name)
                shutil.copyfile(cpath, dst)
                return dst
        except Exception:
            cpath = None
        out = orig(bir_json, tmpdir, neff_name)
        if cpath:
            try:
                os.makedirs(_NEFF_CACHE, exist_ok=True)
                tmp = f"{cpath}.tmp{os.getpid()}"
                shutil.copyfile(out, tmp)
                os.replace(tmp, cpath)
            except Exception:
                pass
        return out

    cached._neff_cached = True
    bass2jax.compile_bir_kernel = cached


_install_neff_cache()


def _in_clean_thread(fn):
    """Run fn on a fresh thread so the Python stack (which bass embeds as
    ant_traceback debug info in the BIR) contains no caller frames -- the
    BIR, and therefore the NEFF cache key, become independent of whichever
    script invoked kernel()."""
    res = {}

    def runner():
        try:
            res["v"] = fn()
        except BaseException as e:  # propagate to caller
            res["e"] = e

    t = threading.Thread(target=runner, name="convkernel")
    t.start()
    t.join()
    if "e" in res:
        raise res["e"]
    return res["v"]


NCORES = 8
CIN = 256
COUT = 256
H = 512
W = 512
KH = KW = 3
PC = 128                 # partition chunk
NCI = CIN // PC          # 2 input-channel chunks
NCO = COUT // PC         # 2 output-channel chunks
HB = H // NCORES         # 64 output rows per core
HIN = HB + 2             # 66 input rows incl. halo
WP = W + 2               # 514 padded width
NTAP = KH * KW
XSPLITS = (3, 7, HIN - 10)   # input-row DMA pieces: head / mid / tail

_nc_cache = {}


def _build(repeats=1):
    nc = bacc.Bacc("TRN2", target_bir_lowering=False, debug=False,
                   num_devices=NCORES)
    xs = nc.dram_tensor("xs", [CIN, HIN, WP], f16, kind="ExternalInput").ap()
    wt = nc.dram_tensor("wt", [NTAP, CIN, COUT], f16, kind="ExternalInput").ap()
    out = nc.dram_tensor("out", [COUT, HB, W], f32, kind="ExternalOutput").ap()

    with tile.TileContext(nc) as tc:
        with tc.tile_pool(name="wpool", bufs=1) as wpool, \
             tc.tile_pool(name="xpool", bufs=1) as xpool, \
             tc.tile_pool(name="opool", bufs=8) as opool, \
             tc.tile_pool(name="pspool", bufs=8, space="PSUM") as pspool:

            # Warm the PE clock gate (HAM) with throwaway matmuls on a
            # memset tile while the input DMAs are in flight, so the real
            # matmul stream starts at 2.4 GHz instead of 1.2.
            warm_src = wpool.tile([PC, PC], f16, name="warm_src")
            nc.vector.memset(warm_src[:], 0.0)
            warm_ps = pspool.tile([PC, PC], f32, tag="ps", name="warm_ps")
            for i in range(60):
                nc.tensor.matmul(warm_ps[:], warm_src[:], warm_src[:],
                                 start=True, stop=True)

            # Weights [128 ci, 9 tap, 2 ci-chunk, 256 co] fp16. DMA issue
            # order tracks the first row-group's consumption order (c-outer
            # matmuls): w[c0,co0], rows(c0), w[c1,co0], rows(c1), w[co1],
            # then the bulk input -- the first matmul gates on ~0.7 MB.
            w_all = wpool.tile([PC, NTAP, NCI, COUT], f16, name="w_all")
            wt_r = wt.rearrange("t (c p) o -> p t c o", p=PC)

            x_sb = [[] for _ in range(NCI)]  # x_sb[c] = [(tile, row0, nrows)]
            r0s = np.cumsum((0,) + XSPLITS)

            def load_x_piece(c, i):
                r0, nr = int(r0s[i]), XSPLITS[i]
                xt = xpool.tile([PC, nr, WP], f16, tag=f"x{c}_{i}",
                                name=f"x{c}_{i}")
                nc.sync.dma_start(
                    xt[:], xs[c * PC:(c + 1) * PC, r0:r0 + nr, :])
                x_sb[c].append((xt, r0, nr))

            nc.sync.dma_start(w_all[:, :, 0, 0:PC], wt_r[:, :, 0, 0:PC])
            load_x_piece(0, 0)
            nc.sync.dma_start(w_all[:, :, 1, 0:PC], wt_r[:, :, 1, 0:PC])
            load_x_piece(1, 0)
            nc.sync.dma_start(w_all[:, :, :, PC:COUT], wt_r[:, :, :, PC:COUT])
            for i in (1, 2):
                for c in range(NCI):
                    load_x_piece(c, i)

            def row_ap(c, rr):
                for xt, r0, nr in x_sb[c]:
                    if rr < r0 + nr:
                        return xt[:, rr - r0, :]
                raise AssertionError(rr)

            for _rep in range(repeats):
                for h in range(HB):
                    for co in range(NCO):
                        ps = pspool.tile([PC, W], f32, tag="ps",
                                         name=f"ps_{h}_{co}")
                        idx = 0
                        for c in range(NCI):
                            for t in range(NTAP):
                                kh, kw = divmod(t, KW)
                                nc.tensor.matmul(
                                    ps[:],
                                    w_all[:, t, c, co * PC:(co + 1) * PC],
                                    row_ap(c, h + kh)[:, kw:kw + W],
                                    start=(idx == 0),
                                    stop=(idx == NTAP * NCI - 1))
                                idx += 1
                        o_t = opool.tile([PC, W], f32, tag="orow",
                                         name=f"o_{h}_{co}")
                        nc.vector.tensor_copy(o_t[:], ps[:])
                        nc.sync.dma_start(
                            out[co * PC:(co + 1) * PC, h, :], o_t[:])
    nc.compile()
    return nc


def _get_nc(repeats=1):
    if repeats not in _nc_cache:
        _nc_cache[repeats] = _in_clean_thread(lambda: _build(repeats))
    return _nc_cache[repeats]


def _make_in_maps(x, weight):
    x_f16 = np.asarray(x[0], dtype=np.float16)
    x_pad = np.pad(x_f16, ((0, 0), (1, 1), (1, 1)))
    w_t = np.ascontiguousarray(
        weight.transpose(2, 3, 1, 0).reshape(NTAP, CIN, COUT).astype(
            np.float16))
    in_maps = []
    for core in range(NCORES):
        xsl = np.ascontiguousarray(x_pad[:, core * HB:core * HB + HIN, :])
        in_maps.append({"xs": xsl, "wt": w_t})
    return in_maps


def kernel(x, weight):
    x = np.asarray(x, dtype=np.float32)
    weight = np.asarray(weight, dtype=np.float32)
    nc = _get_nc(1)
    in_maps = _make_in_maps(x, weight)
    res = _in_clean_thread(lambda: run_bass_kernel_spmd(
        nc, in_maps, core_ids=list(range(NCORES))))
    parts = [res.results[c]["out"] for c in range(NCORES)]
    full = np.concatenate(parts, axis=1)          # [COUT, H, W]
    return full[None].astype(np.float32)
